# revision 3
# baseline (speedup 1.0000x reference)
"""Trainium2 Bass kernel for nn_ChebychevInput.

out[b,o,s] = sum_{i,p} (MAG*coef[o,i,p]) * cos(p*arccos(x[b,i,s])),  p = 0..256

Device pipeline per core (s-shard of 16384 samples, both batches):
  theta stage (flat [96,1024], row = 48b + 16i + sc):
      theta = pi/2 - arctan(x/sqrt(1-x^2))
      th16  = int16(theta * 2^17/(2pi))          # angle in 2^16 half-units
  per (b, sc-chunk of 1024):
      DMA 3 theta rows -> tmp[1, 3072] i16; GPSIMD partition_broadcast
        -> th_bc[128, 3072] i16 (one 1024-col section per input i)
      DVE x6: y32[:, kt*1024:+1024] = int32(th_bc_section * (p/2) + 16384)
        per-partition p/2 from pc[:, kt]; kt k-tile covers p=(kt%2)*128+r+1,
        i=kt//2 (6 k-tiles x 128 rows = p 1..256, i-pure)
      ACT one Sin over low halfwords of y32: tm[128, 6144] f16 = cos(p*theta)
      PE per (m, half): 6 accumulating matmuls [128k,128o]x[128k,512s] -> PSUM
      evac + p=0 term: out_sb = psum + bias[:, m]  (3 tiles on DVE, 1 on ACT)
      DMA out_sb -> out[b, m*128:+128, sc*1024+half*512:+512]
"""
import sys

sys.path.insert(0, "/opt/trn_rl_repo")

import numpy as np

BATCH = 2
INPUT_DIM = 3
N_SAMPLES = 131072
OUTPUT_DIM = 256
POLY_DEGREE = 256  # p = 0..256
N_CORES = 8
S_SHARD = N_SAMPLES // N_CORES  # 16384
SC = 1024                       # sample chunk
NSC = S_SHARD // SC             # 16
NKT = 6                         # k-tiles of 128 rows: p = 1..256, i = kt//2
WEIGHT_MAGNITUDE = float(np.sqrt(6.0 / (INPUT_DIM * (POLY_DEGREE + 1))))
TWO16 = 65536.0

_compiled = {}


def _build():
    import concourse.tile as tile
    from concourse import bacc, mybir

    F32 = mybir.dt.float32
    F16 = mybir.dt.float16
    I32 = mybir.dt.int32
    I16 = mybir.dt.int16
    AF = mybir.ActivationFunctionType
    ALU = mybir.AluOpType

    nc = bacc.Bacc("TRN2", target_bir_lowering=False, debug=False)
    x_d = nc.dram_tensor("x", [BATCH, INPUT_DIM, S_SHARD], F32, kind="ExternalInput")
    w_d = nc.dram_tensor("w", [128, NKT * OUTPUT_DIM], F16, kind="ExternalInput")
    pc_d = nc.dram_tensor("pc", [128, NKT], F32, kind="ExternalInput")
    bias_d = nc.dram_tensor("bias", [128, 2], F32, kind="ExternalInput")
    out_d = nc.dram_tensor("out", [BATCH, OUTPUT_DIM, S_SHARD], F32, kind="ExternalOutput")

    with tile.TileContext(nc) as tc:
        with (
            tc.tile_pool(name="const", bufs=1) as constp,
            tc.tile_pool(name="theta", bufs=1) as thp,
            tc.tile_pool(name="tmp", bufs=3) as tmpp,
            tc.tile_pool(name="bcast", bufs=3) as bcp,
            tc.tile_pool(name="yint", bufs=2) as yp,
            tc.tile_pool(name="tmat", bufs=3) as tp,
            tc.tile_pool(name="outs", bufs=8) as op,
            tc.tile_pool(name="psum", bufs=8, space="PSUM") as pp,
        ):
            w_t = constp.tile([128, NKT * OUTPUT_DIM], F16)
            nc.sync.dma_start(w_t[:], w_d[:])
            pc_t = constp.tile([128, NKT], F32)
            nc.sync.dma_start(pc_t[:], pc_d[:])
            bias_t = constp.tile([128, 2], F32)
            nc.sync.dma_start(bias_t[:], bias_d[:])

            # ---- theta stage: flat [96, 1024]; row = 48*b + 16*i + sc
            xt = thp.tile([96, 1024], F32)
            nc.sync.dma_start(xt[:], x_d[:].rearrange("b i (u c) -> (b i u) c", c=1024))
            sq = thp.tile([96, 1024], F32)
            nc.scalar.activation(sq[:], xt[:], AF.Square)
            r2 = thp.tile([96, 1024], F32)
            nc.scalar.activation(r2[:], sq[:], AF.Sqrt, bias=1.0, scale=-1.0)
            inv = thp.tile([96, 1024], F32)
            nc.vector.reciprocal(inv[:], r2[:])
            q = thp.tile([96, 1024], F32)
            nc.vector.tensor_mul(q[:], xt[:], inv[:])
            asn = thp.tile([96, 1024], F32)
            nc.scalar.activation(asn[:], q[:], AF.Arctan)
            # th16 = int16((pi/2 - asn) * 2^17/(2pi)) = int16(32768 - asn*2^17/2pi)
            th16 = thp.tile([96, 1024], I16)
            nc.vector.tensor_scalar(
                th16[:], asn[:], float(-2.0 * TWO16 / (2 * np.pi)), 32768.0,
                ALU.mult, ALU.add,
            )

            # ---- main loops
            for b in range(BATCH):
                for sc in range(NSC):
                    tmp = tmpp.tile([1, INPUT_DIM * SC], I16)
                    for i in range(INPUT_DIM):
                        row = 48 * b + 16 * i + sc
                        nc.sync.dma_start(tmp[0:1, i * SC:(i + 1) * SC],
                                          th16[row:row + 1, :])
                    th_bc = bcp.tile([128, INPUT_DIM * SC], I16)
                    nc.gpsimd.partition_broadcast(th_bc[:], tmp[:])

                    y32 = yp.tile([128, NKT * SC], I32)
                    for kt in range(NKT):
                        i = kt // 2
                        nc.vector.tensor_scalar(
                            y32[:, kt * SC:(kt + 1) * SC],
                            th_bc[:, i * SC:(i + 1) * SC],
                            pc_t[:, kt:kt + 1], 0.25 * TWO16, ALU.mult, ALU.add,
                        )
                    tm = tp.tile([128, NKT * SC], F16)
                    yv = y32[:].bitcast(I16).rearrange("p (n two) -> p n two", two=2)[:, :, 0]
                    nc.scalar.activation(tm[:], yv, AF.Sin, scale=float(2 * np.pi / TWO16))

                    for m in range(2):
                        for half in range(2):
                            ps = pp.tile([128, 512], F32)
                            for kt in range(NKT):
                                nc.tensor.matmul(
                                    ps[:],
                                    w_t[:, kt * OUTPUT_DIM + m * 128: kt * OUTPUT_DIM + m * 128 + 128],
                                    tm[:, kt * SC + half * 512: kt * SC + half * 512 + 512],
                                    start=(kt == 0), stop=(kt == NKT - 1),
                                )
                            ob = op.tile([128, 512], F32)
                            if m == 1 and half == 1:
                                nc.scalar.activation(ob[:], ps[:], AF.Identity,
                                                     bias=bias_t[:, m:m + 1])
                            else:
                                nc.vector.tensor_scalar(
                                    ob[:], ps[:], bias_t[:, m:m + 1], None, ALU.add)
                            nc.sync.dma_start(
                                out_d[b, m * 128:(m + 1) * 128,
                                      sc * SC + half * 512: sc * SC + half * 512 + 512],
                                ob[:],
                            )
    nc.compile()
    return nc


def _host_prep(coefficients):
    w = (np.asarray(coefficients, dtype=np.float64) * WEIGHT_MAGNITUDE).astype(np.float32)
    # wk[r, kt*256 + o] = w[o, kt//2, (kt%2)*128 + r + 1]
    wk = np.empty((128, NKT * OUTPUT_DIM), np.float32)
    for kt in range(NKT):
        i = kt // 2
        p0 = (kt % 2) * 128 + 1
        wk[:, kt * OUTPUT_DIM:(kt + 1) * OUTPUT_DIM] = w[:, i, p0:p0 + 128].T
    r = np.arange(128)
    pc = np.empty((128, NKT), np.float32)
    for kt in range(NKT):
        pc[:, kt] = ((kt % 2) * 128 + r + 1) * 0.5
    # bias[o', m] = sum_i w[m*128+o', i, 0]
    bias = np.ascontiguousarray(w[:, :, 0].sum(axis=1).reshape(2, 128).T.astype(np.float32))
    return wk.astype(np.float16), pc, bias


def _get_nc():
    if "nc" not in _compiled:
        _compiled["nc"] = _build()
    return _compiled["nc"]


def _build_callable(nc, n_cores=N_CORES):
    """jit(shard_map(bass_exec)) over the first n_cores devices, mirroring
    run_bass_via_pjrt's lowering; inputs must be device_put with the
    returned sharding (axis 0 = per-core concat)."""
    import jax
    from jax.sharding import Mesh, PartitionSpec, NamedSharding
    from jax.experimental.shard_map import shard_map
    from concourse import mybir
    from concourse.bass2jax import (
        _bass_exec_p, install_neuronx_cc_hook, partition_id_tensor)

    install_neuronx_cc_hook()
    partition_name = nc.partition_id_tensor.name if nc.partition_id_tensor else None

    in_names, out_names, out_avals = [], [], []
    for alloc in nc.m.functions[0].allocations:
        if not isinstance(alloc, mybir.MemoryLocationSet):
            continue
        name = alloc.memorylocations[0].name
        if alloc.kind == "ExternalInput":
            if name != partition_name:
                in_names.append(name)
        elif alloc.kind == "ExternalOutput":
            out_names.append(name)
            out_avals.append(jax.core.ShapedArray(
                tuple(alloc.tensor_shape), mybir.dt.np(alloc.dtype)))
    n_params = len(in_names)
    n_outs = len(out_names)
    all_in_names = in_names + out_names
    if partition_name is not None:
        all_in_names.append(partition_name)

    def _body(*args):
        operands = list(args)
        if partition_name is not None:
            operands.append(partition_id_tensor())
        outs = _bass_exec_p.bind(
            *operands,
            out_avals=tuple(out_avals),
            in_names=tuple(all_in_names),
            out_names=tuple(out_names),
            lowering_input_output_aliases=(),
            sim_require_finite=True,
            sim_require_nnan=True,
            nc=nc,
        )
        return tuple(outs)

    devices = jax.devices()[:n_cores]
    mesh = Mesh(np.asarray(devices), ("core",))
    fn = jax.jit(shard_map(
        _body, mesh=mesh,
        in_specs=(PartitionSpec("core"),) * (n_params + n_outs),
        out_specs=(PartitionSpec("core"),) * n_outs, check_rep=False))
    return fn, NamedSharding(mesh, PartitionSpec("core")), in_names, out_avals


def _prep_globals(x, coefficients):
    """Per-core inputs concatenated along axis 0 (core-major)."""
    wk, pc, bias = _host_prep(coefficients)
    xg = np.ascontiguousarray(
        np.asarray(x, dtype=np.float32).reshape(BATCH, INPUT_DIM, N_CORES, S_SHARD)
        .transpose(2, 0, 1, 3).reshape(N_CORES * BATCH, INPUT_DIM, S_SHARD))
    wg = np.tile(wk, (N_CORES, 1))
    pcg = np.tile(pc, (N_CORES, 1))
    biasg = np.tile(bias, (N_CORES, 1))
    return {"x": xg, "w": wg, "pc": pcg, "bias": biasg}


def kernel(x, coefficients):
    from concourse import bass2jax

    nc = _get_nc()
    wk, pc, bias = _host_prep(coefficients)
    x = np.asarray(x, dtype=np.float32)
    in_maps = [
        {"x": np.ascontiguousarray(x[:, :, c * S_SHARD:(c + 1) * S_SHARD]),
         "w": wk, "pc": pc, "bias": bias}
        for c in range(N_CORES)
    ]
    results = bass2jax.run_bass_via_pjrt(nc, in_maps, n_cores=N_CORES)
    out = np.concatenate([results[c]["out"] for c in range(N_CORES)], axis=2)
    return np.ascontiguousarray(out.astype(np.float32))


# revision 5
# speedup vs baseline: 1.8936x; 1.8936x over previous
"""Trainium2 Bass kernel for nn_ChebychevInput.

out[b,o,s] = sum_{i,p} (MAG*coef[o,i,p]) * cos(p*arccos(x[b,i,s])),  p = 0..256

Device pipeline per core (s-shard of 16384 samples, both batches):
  theta stage (flat [96,1024], row = 48b + 16i + sc):
      theta = pi/2 - arctan(x/sqrt(1-x^2))
      th16  = int16(theta * 2^17/(2pi))          # angle in 2^16 half-units
  per (b, sc-chunk of 1024):
      DMA 3 theta rows -> tmp[1, 3072] i16; GPSIMD partition_broadcast
        -> th_bc[128, 3072] i16 (one 1024-col section per input i)
      DVE x6: y32[:, kt*1024:+1024] = int32(th_bc_section * (p/2) + 16384)
        per-partition p/2 from pc[:, kt]; kt k-tile covers p=(kt%2)*128+r+1,
        i=kt//2 (6 k-tiles x 128 rows = p 1..256, i-pure)
      ACT one Sin over low halfwords of y32: tm[128, 6144] f16 = cos(p*theta)
      PE per (m, half): 6 accumulating matmuls [128k,128o]x[128k,512s] -> PSUM
      evac + p=0 term: out_sb = psum + bias[:, m]  (3 tiles on DVE, 1 on ACT)
      DMA out_sb -> out[b, m*128:+128, sc*1024+half*512:+512]
"""
import sys

sys.path.insert(0, "/opt/trn_rl_repo")

import numpy as np

BATCH = 2
INPUT_DIM = 3
N_SAMPLES = 131072
OUTPUT_DIM = 256
POLY_DEGREE = 256  # p = 0..256
N_CORES = 8
S_SHARD = N_SAMPLES // N_CORES  # 16384
SC = 1024                       # sample chunk
NSC = S_SHARD // SC             # 16
NKT = 6                         # k-tiles of 128 rows: p = 1..256, i = kt//2
WEIGHT_MAGNITUDE = float(np.sqrt(6.0 / (INPUT_DIM * (POLY_DEGREE + 1))))
TWO16 = 65536.0

_compiled = {}


def _build(loop_n=1):
    import concourse.tile as tile
    from concourse import bacc, mybir

    F32 = mybir.dt.float32
    F16 = mybir.dt.float16
    I32 = mybir.dt.int32
    I16 = mybir.dt.int16
    AF = mybir.ActivationFunctionType
    ALU = mybir.AluOpType

    nc = bacc.Bacc("TRN2", target_bir_lowering=False, debug=False)
    x_d = nc.dram_tensor("x", [BATCH, INPUT_DIM, S_SHARD], F32, kind="ExternalInput")
    w_d = nc.dram_tensor("w", [128, NKT * OUTPUT_DIM], F16, kind="ExternalInput")
    pc_d = nc.dram_tensor("pc", [128, NKT], F32, kind="ExternalInput")
    bias_d = nc.dram_tensor("bias", [128, 2], F32, kind="ExternalInput")
    out_d = nc.dram_tensor("out", [BATCH, OUTPUT_DIM, S_SHARD], F32, kind="ExternalOutput")

    with tile.TileContext(nc) as tc:
        with (
            tc.tile_pool(name="const", bufs=1) as constp,
            tc.tile_pool(name="theta", bufs=1) as thp,
            tc.tile_pool(name="tmp", bufs=3) as tmpp,
            tc.tile_pool(name="bcast", bufs=3) as bcp,
            tc.tile_pool(name="yint", bufs=2) as yp,
            tc.tile_pool(name="tmat", bufs=3) as tp,
            tc.tile_pool(name="outs", bufs=8) as op,
            tc.tile_pool(name="psum", bufs=8, space="PSUM") as pp,
        ):
            w_t = constp.tile([128, NKT * OUTPUT_DIM], F16)
            nc.sync.dma_start(w_t[:], w_d[:])
            pc_t = constp.tile([128, NKT], F32)
            nc.sync.dma_start(pc_t[:], pc_d[:])
            bias_t = constp.tile([128, 2], F32)
            nc.sync.dma_start(bias_t[:], bias_d[:])

            def body():
                # ---- theta stage: flat [96, 1024]; row = 48*b + 16*i + sc
                xt = thp.tile([96, 1024], F32)
                nc.sync.dma_start(xt[:], x_d[:].rearrange("b i (u c) -> (b i u) c", c=1024))
                sq = thp.tile([96, 1024], F32)
                nc.scalar.activation(sq[:], xt[:], AF.Square)
                r2 = thp.tile([96, 1024], F32)
                nc.scalar.activation(r2[:], sq[:], AF.Sqrt, bias=1.0, scale=-1.0)
                inv = thp.tile([96, 1024], F32)
                nc.vector.reciprocal(inv[:], r2[:])
                q = thp.tile([96, 1024], F32)
                nc.vector.tensor_mul(q[:], xt[:], inv[:])
                asn = thp.tile([96, 1024], F32)
                nc.scalar.activation(asn[:], q[:], AF.Arctan)
                # th16 = int16((pi/2 - asn) * 2^17/(2pi)) = int16(32768 - asn*2^17/2pi)
                th16 = thp.tile([96, 1024], I16)
                nc.vector.tensor_scalar(
                    th16[:], asn[:], float(-2.0 * TWO16 / (2 * np.pi)), 32768.0,
                    ALU.mult, ALU.add,
                )

                # ---- main loops: broadcast per (b, sc-pair), compute per sc
                for b in range(BATCH):
                    for g in range(NSC // 2):
                        tmp = tmpp.tile([1, 2 * INPUT_DIM * SC], I16)
                        for c in range(2):
                            for i in range(INPUT_DIM):
                                row = 48 * b + 16 * i + 2 * g + c
                                nc.sync.dma_start(
                                    tmp[0:1, (3 * c + i) * SC:(3 * c + i + 1) * SC],
                                    th16[row:row + 1, :])
                        th_bc = bcp.tile([128, 2 * INPUT_DIM * SC], I16)
                        nc.gpsimd.partition_broadcast(th_bc[:], tmp[:])

                        for c in range(2):
                            sc = 2 * g + c
                            y32 = yp.tile([128, NKT * SC], I32)
                            for kt in range(NKT):
                                i = kt // 2
                                nc.vector.tensor_scalar(
                                    y32[:, kt * SC:(kt + 1) * SC],
                                    th_bc[:, (3 * c + i) * SC:(3 * c + i + 1) * SC],
                                    pc_t[:, kt:kt + 1], 0.25 * TWO16, ALU.mult, ALU.add,
                                )
                            tm = tp.tile([128, NKT * SC], F16)
                            yv = y32[:].bitcast(I16).rearrange(
                                "p (n two) -> p n two", two=2)[:, :, 0]
                            nc.scalar.activation(tm[:], yv, AF.Sin,
                                                 scale=float(2 * np.pi / TWO16))

                            for m in range(2):
                                for half in range(2):
                                    ps = pp.tile([128, 512], F32)
                                    for kt in range(NKT):
                                        nc.tensor.matmul(
                                            ps[:],
                                            w_t[:, kt * OUTPUT_DIM + m * 128: kt * OUTPUT_DIM + m * 128 + 128],
                                            tm[:, kt * SC + half * 512: kt * SC + half * 512 + 512],
                                            start=(kt == 0), stop=(kt == NKT - 1),
                                        )
                                    ob = op.tile([128, 512], F32)
                                    if c == 1 and m == 1 and half == 1:
                                        nc.scalar.activation(ob[:], ps[:], AF.Identity,
                                                             bias=bias_t[:, m:m + 1])
                                    else:
                                        nc.vector.tensor_scalar(
                                            ob[:], ps[:], bias_t[:, m:m + 1], None, ALU.add)
                                    nc.sync.dma_start(
                                        out_d[b, m * 128:(m + 1) * 128,
                                              sc * SC + half * 512: sc * SC + half * 512 + 512],
                                        ob[:],
                                    )

            if loop_n == 1:
                body()
            else:
                with tc.For_i(0, loop_n, 1):
                    body()
    nc.compile()
    return nc


def _host_prep(coefficients):
    w = (np.asarray(coefficients, dtype=np.float64) * WEIGHT_MAGNITUDE).astype(np.float32)
    # wk[r, kt*256 + o] = w[o, kt//2, (kt%2)*128 + r + 1]
    wk = np.empty((128, NKT * OUTPUT_DIM), np.float32)
    for kt in range(NKT):
        i = kt // 2
        p0 = (kt % 2) * 128 + 1
        wk[:, kt * OUTPUT_DIM:(kt + 1) * OUTPUT_DIM] = w[:, i, p0:p0 + 128].T
    r = np.arange(128)
    pc = np.empty((128, NKT), np.float32)
    for kt in range(NKT):
        pc[:, kt] = ((kt % 2) * 128 + r + 1) * 0.5
    # bias[o', m] = sum_i w[m*128+o', i, 0]
    bias = np.ascontiguousarray(w[:, :, 0].sum(axis=1).reshape(2, 128).T.astype(np.float32))
    return wk.astype(np.float16), pc, bias


def _get_nc(loop_n=1):
    key = ("nc", loop_n)
    if key not in _compiled:
        _compiled[key] = _build(loop_n)
    return _compiled[key]


def _build_callable(nc, n_cores=N_CORES):
    """jit(shard_map(bass_exec)) over the first n_cores devices, mirroring
    run_bass_via_pjrt's lowering; inputs must be device_put with the
    returned sharding (axis 0 = per-core concat)."""
    import jax
    from jax.sharding import Mesh, PartitionSpec, NamedSharding
    from jax.experimental.shard_map import shard_map
    from concourse import mybir
    from concourse.bass2jax import (
        _bass_exec_p, install_neuronx_cc_hook, partition_id_tensor)

    install_neuronx_cc_hook()
    partition_name = nc.partition_id_tensor.name if nc.partition_id_tensor else None

    in_names, out_names, out_avals = [], [], []
    for alloc in nc.m.functions[0].allocations:
        if not isinstance(alloc, mybir.MemoryLocationSet):
            continue
        name = alloc.memorylocations[0].name
        if alloc.kind == "ExternalInput":
            if name != partition_name:
                in_names.append(name)
        elif alloc.kind == "ExternalOutput":
            out_names.append(name)
            out_avals.append(jax.core.ShapedArray(
                tuple(alloc.tensor_shape), mybir.dt.np(alloc.dtype)))
    n_params = len(in_names)
    n_outs = len(out_names)
    all_in_names = in_names + out_names
    if partition_name is not None:
        all_in_names.append(partition_name)

    def _body(*args):
        operands = list(args)
        if partition_name is not None:
            operands.append(partition_id_tensor())
        outs = _bass_exec_p.bind(
            *operands,
            out_avals=tuple(out_avals),
            in_names=tuple(all_in_names),
            out_names=tuple(out_names),
            lowering_input_output_aliases=(),
            sim_require_finite=True,
            sim_require_nnan=True,
            nc=nc,
        )
        return tuple(outs)

    devices = jax.devices()[:n_cores]
    mesh = Mesh(np.asarray(devices), ("core",))
    fn = jax.jit(shard_map(
        _body, mesh=mesh,
        in_specs=(PartitionSpec("core"),) * (n_params + n_outs),
        out_specs=(PartitionSpec("core"),) * n_outs, check_rep=False))
    return fn, NamedSharding(mesh, PartitionSpec("core")), in_names, out_avals


def _prep_globals(x, coefficients):
    """Per-core inputs concatenated along axis 0 (core-major)."""
    wk, pc, bias = _host_prep(coefficients)
    xg = np.ascontiguousarray(
        np.asarray(x, dtype=np.float32).reshape(BATCH, INPUT_DIM, N_CORES, S_SHARD)
        .transpose(2, 0, 1, 3).reshape(N_CORES * BATCH, INPUT_DIM, S_SHARD))
    wg = np.tile(wk, (N_CORES, 1))
    pcg = np.tile(pc, (N_CORES, 1))
    biasg = np.tile(bias, (N_CORES, 1))
    return {"x": xg, "w": wg, "pc": pcg, "bias": biasg}


def kernel(x, coefficients):
    from concourse import bass2jax

    nc = _get_nc()
    wk, pc, bias = _host_prep(coefficients)
    x = np.asarray(x, dtype=np.float32)
    in_maps = [
        {"x": np.ascontiguousarray(x[:, :, c * S_SHARD:(c + 1) * S_SHARD]),
         "w": wk, "pc": pc, "bias": bias}
        for c in range(N_CORES)
    ]
    results = bass2jax.run_bass_via_pjrt(nc, in_maps, n_cores=N_CORES)
    out = np.concatenate([results[c]["out"] for c in range(N_CORES)], axis=2)
    return np.ascontiguousarray(out.astype(np.float32))
